# revision 7
# baseline (speedup 1.0000x reference)
"""Trainium2 Bass kernel for nn_VectorQuantizer (weight-normed VQ bottleneck).

Computes, for x (16, 512, 4096):
  W_in  = weight_norm(in_v, in_g)          (E=256, D=512)
  x_proj = W_in @ x + in_b                  per batch, (E, T)
  xn, cn = l2-normalized x_proj rows / codebook rows
  dist   = |xn|^2 - 2 xn.cn_k + |cn_k|^2 ; logits = -dist
  idx    = argmax(logits) ; xq = codebook[idx]
  out    = W_out @ xq + out_b
  commitment_loss = codebook_loss = mean((x_proj - xq)^2, per batch)

Strategy: data-parallel over batch (2 batches per core, 8 cores). Device does
the two O(N*K*E)-class fp32 contractions on the PE (stage-1 projection,
stage-2 code scores) plus argmax / gather; the out-projection is folded into a
host-precomputed table (out_col = (W_out @ codebook.T + out_b)[idx]) so the
device only gathers rows. Per-position stats (n2, max logit, idx) are packed
and the tiny loss means finish on host in fp64.
"""

import os
import numpy as np

B, D, E, K, T = 16, 512, 256, 1024, 4096
NCORES = 8
BLOC = B // NCORES          # batches per core
EPS = 1e-12

LAST_RESULTS = None         # BassKernelResults of the most recent run (for test.py)

_CACHE = {}


def _maybe_register_ntff_hook():
    """Best-effort registration of the axon NTFF profile hook (trace runs)."""
    try:
        import antenv.axon_hooks  # noqa: F401
        return
    except ImportError:
        pass
    try:
        import sys, types
        import trn_agent_boot.trn_boot as tb
        hook = tb._ntff_profile_via_ctypes("/opt/axon/libaxon_pjrt.so")
        mod = types.ModuleType("antenv.axon_hooks")
        _h = [hook]
        mod.get_axon_ntff_profile_hook = lambda: _h[0]
        mod.set_axon_ntff_profile_hook = lambda h: _h.__setitem__(0, h)
        sys.modules["antenv.axon_hooks"] = mod
        import antenv
        antenv.axon_hooks = mod
    except Exception:
        pass


def build_nc(bloc=BLOC, t_len=T):
    """Build the SPMD single-core program (same program on all 8 cores)."""
    import concourse.bass as bass
    from concourse import bacc
    import concourse.tile as tile
    import concourse.mybir as mybir
    from concourse.masks import make_identity

    dt = mybir.dt
    f32 = dt.float32
    n_pos = bloc * t_len
    n_tiles = n_pos // 512          # 512-position outer tiles
    t_cols = t_len // 512           # 512-col tiles per batch

    nc = bacc.Bacc("TRN2", debug=False, num_devices=NCORES)

    x_d = nc.declare_dram_parameter("x", [bloc, D, t_len], f32, isOutput=False)
    w_d = nc.declare_dram_parameter("w", [D, E], f32, isOutput=False)        # W_in^T
    inb_d = nc.declare_dram_parameter("inb", [E], f32, isOutput=False)
    cn_d = nc.declare_dram_parameter("cn", [E, K], f32, isOutput=False)      # (2*cn)^T
    tbl_d = nc.declare_dram_parameter("tbl", [K, D], f32, isOutput=False)    # W_out@cb_k + out_b
    lg_d = nc.declare_dram_parameter("logits", [n_pos, K], f32, isOutput=True)
    ot_d = nc.declare_dram_parameter("outT", [n_pos, D], f32, isOutput=True)
    # packed per-position stats: cols 0-3 idx bits (per tj), 4-7 lmax, 8-11 n2
    st_d = nc.declare_dram_parameter("stats", [n_tiles, 128, 12], f32, isOutput=True)

    Alu = mybir.AluOpType
    Act = mybir.ActivationFunctionType

    with tile.TileContext(nc) as tc:
        with (
            tc.tile_pool(name="const", bufs=1) as cpool,
            tc.tile_pool(name="xin", bufs=4) as xpool,
            tc.tile_pool(name="xf", bufs=3) as xfpool,
            tc.tile_pool(name="lg", bufs=3) as lgpool,
            tc.tile_pool(name="gt", bufs=3) as gtpool,
            tc.tile_pool(name="sc", bufs=6) as scpool,
            tc.tile_pool(name="st", bufs=3) as stpool,
            tc.tile_pool(name="mx", bufs=4) as mxpool,
            tc.tile_pool(name="junk", bufs=2) as jpool,
            tc.tile_pool(name="psE", bufs=2, space="PSUM") as pEpool,
            tc.tile_pool(name="psT", bufs=2, space="PSUM") as pTpool,
            tc.tile_pool(name="psR", bufs=2, space="PSUM") as pRpool,
        ):
            # ---- persistent constants ----
            w_sb = cpool.tile([128, 4 * E], f32)            # W_in^T, chunk c at [:, c*E:(c+1)*E]
            for c in range(4):
                nc.gpsimd.dma_start(w_sb[:, c * E:(c + 1) * E], w_d[c * 128:(c + 1) * 128, :])
            cn_sb = cpool.tile([128, 2 * K], f32)           # (2cn)^T, chunk m at [:, m*K:(m+1)*K]
            for m in range(2):
                nc.gpsimd.dma_start(cn_sb[:, m * K:(m + 1) * K], cn_d[m * 128:(m + 1) * 128, :])
            inb_sb = cpool.tile([128, 2], f32)              # in_b as two [128,1] columns
            for m in range(2):
                nc.gpsimd.dma_start(inb_sb[:, m:m + 1], inb_d[m * 128:(m + 1) * 128])
            id_sb = cpool.tile([128, 128], f32)
            make_identity(nc, id_sb[:])

            for it in range(n_tiles):
                b, tcol = divmod(it, t_cols)
                t0 = tcol * 512

                # ---- load x tile: 4 D-chunks of [128, 512] in one DMA ----
                x_t = xpool.tile([128, 4, 512], f32)
                for c in range(4):
                    nc.sync.dma_start(
                        x_t[:, c, :], x_d[b, c * 128:(c + 1) * 128, t0:t0 + 512])

                # ---- stage 1: x_proj chunks (E on partitions, 512 positions free) ----
                xf_t = xfpool.tile([128, 2, 512], f32)
                for m in range(2):
                    pE = pEpool.tile([128, 512], f32)
                    for c in range(4):
                        nc.tensor.matmul(
                            pE[:],
                            w_sb[:, c * E + m * 128: c * E + (m + 1) * 128],
                            x_t[:, c, :],
                            start=(c == 0), stop=(c == 3),
                        )
                    # evacuate + bias add (in_b per E-row = per partition)
                    nc.vector.tensor_scalar_add(xf_t[:, m, :], pE[:], inb_sb[:, m:m + 1])

                lg = lgpool.tile([128, 4, K], f32)
                gt = gtpool.tile([128, 4, D], f32)
                st = stpool.tile([128, 12], f32)
                for tj in range(4):
                    sc = scpool.tile([128, 4], f32)
                    # ---- per-position squared norms via PE transpose + square-reduce ----
                    pT = pTpool.tile([128, 256], f32)
                    for m in range(2):
                        nc.tensor.transpose(
                            pT[:, m * 128:(m + 1) * 128],
                            xf_t[:, m, tj * 128:(tj + 1) * 128], id_sb[:])
                    junk = jpool.tile([128, 256], f32)
                    nc.scalar.activation(junk[:], pT[:], Act.Square,
                                         accum_out=st[:, 8 + tj:9 + tj])
                    # ---- scalar chain: norm, 1/norm, |xn|^2, -(|xn|^2+1) ----
                    nc.scalar.activation(sc[:, 0:1], st[:, 8 + tj:9 + tj], Act.Sqrt)
                    nc.vector.tensor_scalar_max(sc[:, 0:1], sc[:, 0:1], EPS)
                    nc.vector.reciprocal(sc[:, 1:2], sc[:, 0:1])
                    nc.vector.tensor_scalar(
                        sc[:, 2:3], st[:, 8 + tj:9 + tj], sc[:, 1:2], sc[:, 1:2],
                        op0=Alu.mult, op1=Alu.mult)
                    nc.vector.tensor_scalar(
                        sc[:, 3:4], sc[:, 2:3], 1.0, -1.0,
                        op0=Alu.add, op1=Alu.mult)

                    # ---- stage 2: code scores r2 = xf @ (2cn)^T, then logits ----
                    pR = pRpool.tile([128, K], f32)
                    for kc in range(2):
                        for m in range(2):
                            nc.tensor.matmul(
                                pR[:, kc * 512:(kc + 1) * 512],
                                xf_t[:, m, tj * 128:(tj + 1) * 128],
                                cn_sb[:, m * K + kc * 512: m * K + (kc + 1) * 512],
                                start=(m == 0), stop=(m == 1),
                            )
                    # logits = r2 * (1/norm) - (|xn|^2 + 1)
                    nc.scalar.activation(
                        lg[:, tj, :], pR[:],
                        Act.Identity, bias=sc[:, 3:4], scale=sc[:, 1:2])

                    # ---- argmax over K ----
                    nc.vector.tensor_reduce(
                        st[:, 4 + tj:5 + tj], lg[:, tj, :],
                        axis=mybir.AxisListType.X, op=Alu.max)
                    mx8 = mxpool.tile([128, 8], f32)
                    nc.vector.tensor_copy(mx8[:], st[:, 4 + tj:5 + tj].to_broadcast([128, 8]))
                    ix8 = mxpool.tile([128, 8], dt.uint32, tag="ix8")
                    nc.vector.max_index(ix8[:], mx8[:], lg[:, tj, :])
                    nc.vector.tensor_copy(st[:, tj:tj + 1].bitcast(dt.uint32), ix8[:, 0:1])

                    # ---- gather out-projection rows by idx ----
                    nc.gpsimd.indirect_dma_start(
                        out=gt[:, tj, :], out_offset=None,
                        in_=tbl_d[:],
                        in_offset=bass.IndirectOffsetOnAxis(ap=ix8[:, 0:1], axis=0),
                    )

                    # ---- per-tj outputs so transfers overlap the PE stream ----
                    pos0 = it * 512 + tj * 128
                    nc.sync.dma_start(lg_d[pos0:pos0 + 128, :], lg[:, tj, :])
                    nc.sync.dma_start(ot_d[pos0:pos0 + 128, :], gt[:, tj, :])

                nc.sync.dma_start(st_d[it, :, :], st[:])

    nc.compile()
    return nc


def _get_nc():
    if "nc" not in _CACHE:
        _CACHE["nc"] = build_nc()
    return _CACHE["nc"]


def kernel(x, in_v, in_g, in_b, out_v, out_g, out_b, codebook):
    global LAST_RESULTS
    from concourse.bass_utils import run_bass_kernel_spmd

    x = np.asarray(x, np.float32)
    in_v = np.asarray(in_v, np.float64)
    in_g = np.asarray(in_g, np.float64)
    in_b64 = np.asarray(in_b, np.float64)
    out_v = np.asarray(out_v, np.float64)
    out_g = np.asarray(out_g, np.float64)
    out_b64 = np.asarray(out_b, np.float64)
    cb = np.asarray(codebook, np.float64)

    # host precompute (small, fp64 then round)
    W_in = in_g[:, None] * in_v / np.linalg.norm(in_v, axis=1, keepdims=True)      # (E, D)
    W_out = out_g[:, None] * out_v / np.linalg.norm(out_v, axis=1, keepdims=True)  # (D, E)
    cn = cb / np.maximum(np.linalg.norm(cb, axis=1, keepdims=True), EPS)           # (K, E)
    w_inT = np.ascontiguousarray(W_in.T).astype(np.float32)                        # (D, E)
    cn2T = np.ascontiguousarray((2.0 * cn).T).astype(np.float32)                   # (E, K)
    tbl = (cb @ W_out.T + out_b64[None, :]).astype(np.float32)                     # (K, D)
    inb32 = in_b64.astype(np.float32)
    cbn2 = (cb ** 2).sum(axis=1)                                                   # (K,)
    cbn = np.sqrt(cbn2)

    in_maps = []
    for c in range(NCORES):
        in_maps.append({
            "x": np.ascontiguousarray(x[c * BLOC:(c + 1) * BLOC]),
            "w": w_inT, "inb": inb32, "cn": cn2T, "tbl": tbl,
        })

    trace = os.environ.get("BASS_TRACE", "") not in ("", "0")
    if trace:
        _maybe_register_ntff_hook()
    nc = _get_nc()
    res = None
    for attempt in range(3):
        try:
            res = run_bass_kernel_spmd(nc, in_maps, list(range(NCORES)), trace=trace and attempt == 0)
            break
        except Exception:
            if attempt == 2:
                raise
            # device may be wedged (e.g. a prior run died mid-NEFF) — reset and retry
            try:
                import ctypes
                lib = ctypes.CDLL("/opt/axon/libaxon_pjrt.so")
                lib.axon_reset.restype = ctypes.c_int64
                lib.axon_reset()
            except Exception:
                pass
    LAST_RESULTS = res

    n_pos = BLOC * T
    logits = np.empty((B * T, K), np.float32)
    out = np.empty((B, D, T), np.float32)
    idxs = np.empty((B, T), np.int32)
    closs = np.empty((B,), np.float32)
    for c in range(NCORES):
        r = res.results[c]
        logits[c * n_pos:(c + 1) * n_pos] = r["logits"]
        outT = r["outT"].reshape(BLOC, T, D)
        out[c * BLOC:(c + 1) * BLOC] = outT.transpose(0, 2, 1)
        stats = r["stats"]                                  # (n_tiles, 128, 12)
        ix = np.ascontiguousarray(
            stats[:, :, 0:4].transpose(0, 2, 1)).reshape(-1).view(np.uint32).astype(np.int64)
        lmax = stats[:, :, 4:8].transpose(0, 2, 1).reshape(-1).astype(np.float64)
        n2 = stats[:, :, 8:12].transpose(0, 2, 1).reshape(-1).astype(np.float64)
        idxs[c * BLOC:(c + 1) * BLOC] = ix.reshape(BLOC, T).astype(np.int32)
        # loss on host in fp64 from device stats
        norm = np.maximum(np.sqrt(n2), EPS)
        xnxn = n2 / (norm * norm)
        r2idx = (lmax + xnxn + 1.0) * norm          # = 2 * xf . cn_idx
        xf_cb = 0.5 * r2idx * cbn[ix]               # = xf . cb_idx
        lossp = (n2 - 2.0 * xf_cb + cbn2[ix]) / (E * T)
        lossb = lossp.reshape(BLOC, T).sum(axis=1)
        closs[c * BLOC:(c + 1) * BLOC] = lossb.astype(np.float32)

    return out, logits, idxs, closs, closs.copy()


# revision 8
# speedup vs baseline: 1.1871x; 1.1871x over previous
"""Trainium2 Bass kernel for nn_VectorQuantizer (weight-normed VQ bottleneck).

Computes, for x (16, 512, 4096):
  W_in  = weight_norm(in_v, in_g)          (E=256, D=512)
  x_proj = W_in @ x + in_b                  per batch, (E, T)
  xn, cn = l2-normalized x_proj rows / codebook rows
  dist   = |xn|^2 - 2 xn.cn_k + |cn_k|^2 ; logits = -dist
  idx    = argmax(logits) ; xq = codebook[idx]
  out    = W_out @ xq + out_b
  commitment_loss = codebook_loss = mean((x_proj - xq)^2, per batch)

Strategy: data-parallel over batch (2 batches per core, 8 cores). Device does
the two O(N*K*E)-class fp32 contractions on the PE (stage-1 projection,
stage-2 code scores) plus argmax / gather; the out-projection is folded into a
host-precomputed table (out_col = (W_out @ codebook.T + out_b)[idx]) so the
device only gathers rows. Per-position stats (n2, max logit, idx) are packed
and the tiny loss means finish on host in fp64.
"""

import os
import numpy as np

B, D, E, K, T = 16, 512, 256, 1024, 4096
NCORES = 8
BLOC = B // NCORES          # batches per core
EPS = 1e-12

LAST_RESULTS = None         # BassKernelResults of the most recent run (for test.py)

_CACHE = {}


def _maybe_register_ntff_hook():
    """Best-effort registration of the axon NTFF profile hook (trace runs)."""
    try:
        import antenv.axon_hooks  # noqa: F401
        return
    except ImportError:
        pass
    try:
        import sys, types
        import trn_agent_boot.trn_boot as tb
        hook = tb._ntff_profile_via_ctypes("/opt/axon/libaxon_pjrt.so")
        mod = types.ModuleType("antenv.axon_hooks")
        _h = [hook]
        mod.get_axon_ntff_profile_hook = lambda: _h[0]
        mod.set_axon_ntff_profile_hook = lambda h: _h.__setitem__(0, h)
        sys.modules["antenv.axon_hooks"] = mod
        import antenv
        antenv.axon_hooks = mod
    except Exception:
        pass


def build_nc(bloc=BLOC, t_len=T):
    """Build the SPMD single-core program (same program on all 8 cores)."""
    import concourse.bass as bass
    from concourse import bacc
    import concourse.tile as tile
    import concourse.mybir as mybir
    from concourse.masks import make_identity

    dt = mybir.dt
    f32 = dt.float32
    n_pos = bloc * t_len
    n_tiles = n_pos // 512          # 512-position outer tiles
    t_cols = t_len // 512           # 512-col tiles per batch

    nc = bacc.Bacc("TRN2", debug=False, num_devices=NCORES)

    x_d = nc.declare_dram_parameter("x", [bloc, D, t_len], f32, isOutput=False)
    w_d = nc.declare_dram_parameter("w", [D, E], f32, isOutput=False)        # W_in^T
    inb_d = nc.declare_dram_parameter("inb", [E], f32, isOutput=False)
    cn_d = nc.declare_dram_parameter("cn", [E, K], f32, isOutput=False)      # (2*cn)^T
    tbl_d = nc.declare_dram_parameter("tbl", [K, D], f32, isOutput=False)    # W_out@cb_k + out_b
    lg_d = nc.declare_dram_parameter("logits", [n_pos, K], f32, isOutput=True)
    ot_d = nc.declare_dram_parameter("outT", [n_pos, D], f32, isOutput=True)
    # packed per-position stats: cols 0-3 idx bits (per tj), 4-7 lmax, 8-11 n2
    st_d = nc.declare_dram_parameter("stats", [n_tiles, 128, 12], f32, isOutput=True)

    Alu = mybir.AluOpType
    Act = mybir.ActivationFunctionType

    with tile.TileContext(nc) as tc:
        with (
            tc.tile_pool(name="const", bufs=1) as cpool,
            tc.tile_pool(name="xin", bufs=3) as xpool,
            tc.tile_pool(name="xf", bufs=3) as xfpool,
            tc.tile_pool(name="lg", bufs=2) as lgpool,
            tc.tile_pool(name="gt", bufs=2) as gtpool,
            tc.tile_pool(name="sc", bufs=6) as scpool,
            tc.tile_pool(name="st", bufs=3) as stpool,
            tc.tile_pool(name="mx", bufs=4) as mxpool,
            tc.tile_pool(name="junk", bufs=2) as jpool,
            tc.tile_pool(name="psE", bufs=2, space="PSUM") as pEpool,
            tc.tile_pool(name="psT", bufs=2, space="PSUM") as pTpool,
            tc.tile_pool(name="psR", bufs=2, space="PSUM") as pRpool,
        ):
            # ---- persistent constants ----
            w_sb = cpool.tile([128, 4 * E], f32)            # W_in^T, chunk c at [:, c*E:(c+1)*E]
            for c in range(4):
                nc.sync.dma_start(w_sb[:, c * E:(c + 1) * E], w_d[c * 128:(c + 1) * 128, :])
            cn_sb = cpool.tile([128, 2 * K], f32)           # (2cn)^T, chunk m at [:, m*K:(m+1)*K]
            for m in range(2):
                nc.sync.dma_start(cn_sb[:, m * K:(m + 1) * K], cn_d[m * 128:(m + 1) * 128, :])
            inb_sb = cpool.tile([128, 2], f32)              # in_b as two [128,1] columns
            for m in range(2):
                nc.sync.dma_start(inb_sb[:, m:m + 1], inb_d[m * 128:(m + 1) * 128])
            id_sb = cpool.tile([128, 128], f32)
            make_identity(nc, id_sb[:])

            for it in range(n_tiles):
                b, tcol = divmod(it, t_cols)
                t0 = tcol * 512

                # ---- load x tile: 4 D-chunks of [128, 512] in one DMA ----
                x_t = xpool.tile([128, 4, 512], f32)
                for c in range(4):
                    nc.sync.dma_start(
                        x_t[:, c, :], x_d[b, c * 128:(c + 1) * 128, t0:t0 + 512])

                # ---- stage 1: x_proj chunks (E on partitions, 512 positions free) ----
                xf_t = xfpool.tile([128, 2, 512], f32)
                for m in range(2):
                    pE = pEpool.tile([128, 512], f32)
                    for c in range(4):
                        nc.tensor.matmul(
                            pE[:],
                            w_sb[:, c * E + m * 128: c * E + (m + 1) * 128],
                            x_t[:, c, :],
                            start=(c == 0), stop=(c == 3),
                        )
                    # evacuate + bias add (in_b per E-row = per partition)
                    nc.vector.tensor_scalar_add(xf_t[:, m, :], pE[:], inb_sb[:, m:m + 1])

                lg = lgpool.tile([128, 4, K], f32)
                gt = gtpool.tile([128, 4, D], f32)
                st = stpool.tile([128, 12], f32)
                for tj in range(4):
                    sc = scpool.tile([128, 4], f32)
                    # ---- per-position squared norms via PE transpose + square-reduce ----
                    pT = pTpool.tile([128, 256], f32)
                    for m in range(2):
                        nc.tensor.transpose(
                            pT[:, m * 128:(m + 1) * 128],
                            xf_t[:, m, tj * 128:(tj + 1) * 128], id_sb[:])
                    junk = jpool.tile([128, 256], f32)
                    nc.scalar.activation(junk[:], pT[:], Act.Square,
                                         accum_out=st[:, 8 + tj:9 + tj])
                    # ---- scalar chain: norm, 1/norm, |xn|^2, -(|xn|^2+1) ----
                    nc.scalar.activation(sc[:, 0:1], st[:, 8 + tj:9 + tj], Act.Sqrt)
                    nc.vector.tensor_scalar_max(sc[:, 0:1], sc[:, 0:1], EPS)
                    nc.vector.reciprocal(sc[:, 1:2], sc[:, 0:1])
                    nc.vector.tensor_scalar(
                        sc[:, 2:3], st[:, 8 + tj:9 + tj], sc[:, 1:2], sc[:, 1:2],
                        op0=Alu.mult, op1=Alu.mult)
                    nc.vector.tensor_scalar(
                        sc[:, 3:4], sc[:, 2:3], 1.0, -1.0,
                        op0=Alu.add, op1=Alu.mult)

                    # ---- stage 2: code scores r2 = xf @ (2cn)^T, then logits ----
                    pR = pRpool.tile([128, K], f32)
                    for kc in range(2):
                        for m in range(2):
                            nc.tensor.matmul(
                                pR[:, kc * 512:(kc + 1) * 512],
                                xf_t[:, m, tj * 128:(tj + 1) * 128],
                                cn_sb[:, m * K + kc * 512: m * K + (kc + 1) * 512],
                                start=(m == 0), stop=(m == 1),
                            )
                    # logits = r2 * (1/norm) - (|xn|^2 + 1)
                    nc.scalar.activation(
                        lg[:, tj, :], pR[:],
                        Act.Identity, bias=sc[:, 3:4], scale=sc[:, 1:2])

                    # ---- argmax over K ----
                    nc.vector.tensor_reduce(
                        st[:, 4 + tj:5 + tj], lg[:, tj, :],
                        axis=mybir.AxisListType.X, op=Alu.max)
                    mx8 = mxpool.tile([128, 8], f32)
                    nc.vector.tensor_copy(mx8[:], st[:, 4 + tj:5 + tj].to_broadcast([128, 8]))
                    ix8 = mxpool.tile([128, 8], dt.uint32, tag="ix8")
                    nc.vector.max_index(ix8[:], mx8[:], lg[:, tj, :])
                    nc.vector.tensor_copy(st[:, tj:tj + 1].bitcast(dt.uint32), ix8[:, 0:1])

                    # ---- gather out-projection rows by idx ----
                    nc.gpsimd.indirect_dma_start(
                        out=gt[:, tj, :], out_offset=None,
                        in_=tbl_d[:],
                        in_offset=bass.IndirectOffsetOnAxis(ap=ix8[:, 0:1], axis=0),
                    )

                    # ---- per-tj outputs so transfers overlap the PE stream ----
                    pos0 = it * 512 + tj * 128
                    nc.sync.dma_start(lg_d[pos0:pos0 + 128, :], lg[:, tj, :])
                    nc.sync.dma_start(ot_d[pos0:pos0 + 128, :], gt[:, tj, :])

                nc.sync.dma_start(st_d[it, :, :], st[:])

    nc.compile()
    return nc


def _get_nc():
    if "nc" not in _CACHE:
        _CACHE["nc"] = build_nc()
    return _CACHE["nc"]


def kernel(x, in_v, in_g, in_b, out_v, out_g, out_b, codebook):
    global LAST_RESULTS
    from concourse.bass_utils import run_bass_kernel_spmd

    x = np.asarray(x, np.float32)
    in_v = np.asarray(in_v, np.float64)
    in_g = np.asarray(in_g, np.float64)
    in_b64 = np.asarray(in_b, np.float64)
    out_v = np.asarray(out_v, np.float64)
    out_g = np.asarray(out_g, np.float64)
    out_b64 = np.asarray(out_b, np.float64)
    cb = np.asarray(codebook, np.float64)

    # host precompute (small, fp64 then round)
    W_in = in_g[:, None] * in_v / np.linalg.norm(in_v, axis=1, keepdims=True)      # (E, D)
    W_out = out_g[:, None] * out_v / np.linalg.norm(out_v, axis=1, keepdims=True)  # (D, E)
    cn = cb / np.maximum(np.linalg.norm(cb, axis=1, keepdims=True), EPS)           # (K, E)
    w_inT = np.ascontiguousarray(W_in.T).astype(np.float32)                        # (D, E)
    cn2T = np.ascontiguousarray((2.0 * cn).T).astype(np.float32)                   # (E, K)
    tbl = (cb @ W_out.T + out_b64[None, :]).astype(np.float32)                     # (K, D)
    inb32 = in_b64.astype(np.float32)
    cbn2 = (cb ** 2).sum(axis=1)                                                   # (K,)
    cbn = np.sqrt(cbn2)

    in_maps = []
    for c in range(NCORES):
        in_maps.append({
            "x": np.ascontiguousarray(x[c * BLOC:(c + 1) * BLOC]),
            "w": w_inT, "inb": inb32, "cn": cn2T, "tbl": tbl,
        })

    trace = os.environ.get("BASS_TRACE", "") not in ("", "0")
    if trace:
        _maybe_register_ntff_hook()
    nc = _get_nc()
    res = None
    for attempt in range(3):
        try:
            res = run_bass_kernel_spmd(nc, in_maps, list(range(NCORES)), trace=trace and attempt == 0)
            break
        except Exception:
            if attempt == 2:
                raise
            # device may be wedged (e.g. a prior run died mid-NEFF) — reset and retry
            try:
                import ctypes
                lib = ctypes.CDLL("/opt/axon/libaxon_pjrt.so")
                lib.axon_reset.restype = ctypes.c_int64
                lib.axon_reset()
            except Exception:
                pass
    LAST_RESULTS = res

    n_pos = BLOC * T
    logits = np.empty((B * T, K), np.float32)
    out = np.empty((B, D, T), np.float32)
    idxs = np.empty((B, T), np.int32)
    closs = np.empty((B,), np.float32)
    for c in range(NCORES):
        r = res.results[c]
        logits[c * n_pos:(c + 1) * n_pos] = r["logits"]
        outT = r["outT"].reshape(BLOC, T, D)
        out[c * BLOC:(c + 1) * BLOC] = outT.transpose(0, 2, 1)
        stats = r["stats"]                                  # (n_tiles, 128, 12)
        ix = np.ascontiguousarray(
            stats[:, :, 0:4].transpose(0, 2, 1)).reshape(-1).view(np.uint32).astype(np.int64)
        lmax = stats[:, :, 4:8].transpose(0, 2, 1).reshape(-1).astype(np.float64)
        n2 = stats[:, :, 8:12].transpose(0, 2, 1).reshape(-1).astype(np.float64)
        idxs[c * BLOC:(c + 1) * BLOC] = ix.reshape(BLOC, T).astype(np.int32)
        # loss on host in fp64 from device stats
        norm = np.maximum(np.sqrt(n2), EPS)
        xnxn = n2 / (norm * norm)
        r2idx = (lmax + xnxn + 1.0) * norm          # = 2 * xf . cn_idx
        xf_cb = 0.5 * r2idx * cbn[ix]               # = xf . cb_idx
        lossp = (n2 - 2.0 * xf_cb + cbn2[ix]) / (E * T)
        lossb = lossp.reshape(BLOC, T).sum(axis=1)
        closs[c * BLOC:(c + 1) * BLOC] = lossb.astype(np.float32)

    return out, logits, idxs, closs, closs.copy()
